# revision 25
# baseline (speedup 1.0000x reference)
"""CGCNN (no BN) message-passing GNN on 8 Trainium2 NeuronCores.

Strategy (self-contained; shapes hardcoded from the problem spec):
 - Nodes are permuted on the host into 392 blocks of 128 slots, balancing
   per-block in-edge counts. Cores own 49 contiguous blocks (6272 slots).
 - Edges are owned by the core that owns their destination block; within a
   block, edges are split by source-slot half (<32768 vs >=32768) so each
   128-edge tile gathers from a single int16-indexable table view, then
   padded to a uniform (TL, TH) tile count per block so all 8 cores run one
   SPMD program.
 - Per 128-edge tile on device: dma_gather (SBUF source, transposed) pulls
   x[src] / x[dst] columns in channel-major bf16; three PE matmuls
   (xi@W1 + xj@W2 + eaT@W3b) accumulate the conv pre-activation in PSUM;
   ACT computes sigmoid/softplus; DVE builds a one-hot dst matrix which PE
   uses to segment-sum messages into the block accumulator.
 - LayerNorm + residual + softplus per block in f32; updated x is written to
   a bf16 local table (for x[dst] gathers) and, between conv layers, an
   in-kernel AllGather replicates every core's slice into the full bf16
   gather table.
 - Gaussian edge smearing, the initial embedding, and the tiny pooled-MLP
   head run on the host in f32 (the MLP must be f32: bf16 there destroys the
   tiny cross-graph spread; it is 0.01% of the FLOPs).
"""

import os as _os
import numpy as np
import ml_dtypes

import concourse.bass as bass
import concourse.tile as tile
from concourse import bacc, mybir
from concourse.bass_utils import run_bass_kernel_spmd

BF16 = ml_dtypes.bfloat16

# Problem constants
N_NODES, N_EDGES, NODE_D, EDGE_D, EMB_D, N_GRAPHS = 50000, 800000, 128, 100, 92, 256
N_CONV, FC_D, N_FC, CUTOFF = 3, 128, 2, 6.0

LAST_RESULTS = None        # BassKernelResults of the most recent run (for tests)
LAST_RERUN_S = None        # wall seconds of a warm re-execution (KERNEL_RERUN=1)

N_CORES = 8
SLOTS = 50176              # 392 blocks * 128
BLOCKS = SLOTS // 128      # 392
NBLK = BLOCKS // N_CORES   # 49 blocks per core
CORE_SLOTS = NBLK * 128    # 6272
LO_SLOTS = 32768           # slots gatherable from the low table view
CHUNK = 2                  # blocks processed per gather chunk


# --------------------------------------------------------------------------
# Device program
# --------------------------------------------------------------------------

def build_nc(TL, TH, nblk=NBLK, ranks=BLOCKS, n_cores=N_CORES,
             lo_ranks=LO_SLOTS // 128, exchange="collective"):
    """Build the SPMD Bass program. TL/TH = low/high tiles per block."""
    TPB = TL + TH                 # tiles per block
    NT = nblk * TPB               # tiles per core
    S = NT * 128                  # edge slots per core
    SLO = nblk * TL * 128
    SHI = nblk * TH * 128
    core_slots = nblk * 128
    f32, bf, i16 = mybir.dt.float32, mybir.dt.bfloat16, mybir.dt.int16

    nc = bacc.Bacc("TRN2", target_bir_lowering=False, debug=False,
                   num_devices=n_cores)

    # inputs
    tab0_d = nc.dram_tensor("tab0", [128, ranks * 128], bf, kind="ExternalInput").ap()
    loc0_d = nc.dram_tensor("loc0", [128, nblk * 128], bf, kind="ExternalInput").ap()
    xloc0_d = nc.dram_tensor("xloc0", [core_slots, 128], f32, kind="ExternalInput").ap()
    eaT_d = nc.dram_tensor("eaT", [101, S], bf, kind="ExternalInput").ap()
    ixi_d = nc.dram_tensor("ixi", [128, S // 16], i16, kind="ExternalInput").ap()
    ixlo_d = nc.dram_tensor("ixlo", [128, SLO // 16], i16, kind="ExternalInput").ap()
    ixhi_d = nc.dram_tensor("ixhi", [128, SHI // 16], i16, kind="ExternalInput").ap()
    dst_d = nc.dram_tensor("dstv", [128, NT], f32, kind="ExternalInput").ap()
    iota_d = nc.dram_tensor("iota", [128, 128], f32, kind="ExternalInput").ap()
    wxi_d = nc.dram_tensor("wxi", [128, N_CONV, 256], bf, kind="ExternalInput").ap()
    wxj_d = nc.dram_tensor("wxj", [128, N_CONV, 256], bf, kind="ExternalInput").ap()
    wea_d = nc.dram_tensor("wea", [101, N_CONV, 256], bf, kind="ExternalInput").ap()
    g_d = nc.dram_tensor("lng", [128, N_CONV, 128], f32, kind="ExternalInput").ap()
    b_d = nc.dram_tensor("lnb", [128, N_CONV, 128], f32, kind="ExternalInput").ap()

    # internal DRAM
    xmast = [
        nc.dram_tensor(f"xmast{i}", [core_slots, 128], f32, kind="Internal").ap()
        for i in range(2)
    ]
    if exchange == "collective":
        xout = [
            nc.dram_tensor(f"xout{i}", [core_slots, 128], bf, kind="Internal").ap()
            for i in range(2)
        ]
        xall = [
            nc.dram_tensor(f"xall{i}", [n_cores * core_slots, 128], bf,
                           kind="Internal", addr_space="Shared").ap()
            for i in range(2)
        ]
    # output
    xfin_d = nc.dram_tensor("xfin", [core_slots, 128], f32, kind="ExternalOutput").ap()

    rg = [list(range(n_cores))]

    with tile.TileContext(nc) as tc:
        with (
            tc.tile_pool(name="persist", bufs=1) as persist,
            tc.tile_pool(name="gxi", bufs=2) as gxi_p,
            tc.tile_pool(name="glo", bufs=2) as glo_p,
            tc.tile_pool(name="ghi", bufs=2) as ghi_p,
            tc.tile_pool(name="eat", bufs=2) as ea_p,
            tc.tile_pool(name="idx", bufs=2) as idx_p,
            tc.tile_pool(name="small", bufs=3) as small_p,
            tc.tile_pool(name="xio", bufs=2) as xio_p,
            tc.tile_pool(name="stats", bufs=2) as stats_p,
            tc.tile_pool(name="zc", bufs=2, space="PSUM") as zc_p,
            tc.tile_pool(name="agg", bufs=4, space="PSUM") as agg_p,
        ):
            # persistent SBUF
            tab_s = persist.tile([128, ranks * 128], bf)
            loc_s = persist.tile([128, nblk, 128], bf)
            dst_s = persist.tile([128, NT], f32)
            iota_s = persist.tile([128, 128], f32)
            wxi_s = persist.tile([128, N_CONV, 256], bf)
            wxj_s = persist.tile([128, N_CONV, 256], bf)
            wea_s = persist.tile([101, N_CONV, 256], bf)
            g_s = persist.tile([128, N_CONV, 128], f32)
            b_s = persist.tile([128, N_CONV, 128], f32)
            eps_s = persist.tile([128, 1], f32)
            ones_s = persist.tile([128, 1], f32)

            nc.sync.dma_start(tab_s[:], tab0_d)
            nc.sync.dma_start(loc_s[:], loc0_d.rearrange("p (r c) -> p r c", c=128))
            nc.sync.dma_start(dst_s[:], dst_d)
            nc.sync.dma_start(iota_s[:], iota_d)
            nc.sync.dma_start(wxi_s[:], wxi_d)
            nc.sync.dma_start(wxj_s[:], wxj_d)
            nc.sync.dma_start(wea_s[:], wea_d)
            nc.sync.dma_start(g_s[:], g_d)
            nc.sync.dma_start(b_s[:], b_d)
            nc.vector.memset(eps_s[:], 1e-5)
            nc.vector.memset(ones_s[:], 1.0)

            n_chunks = (nblk + CHUNK - 1) // CHUNK
            tab_lo_view = tab_s[:, : lo_ranks * 128]
            tab_hi_view = tab_s[:, lo_ranks * 128:]
            loc_flat = loc_s.rearrange("p r c -> p (r c)")

            for layer in range(N_CONV):
                xold_src = xloc0_d if layer == 0 else xmast[layer - 1]
                xnew_dst = xfin_d if layer == N_CONV - 1 else xmast[layer]

                for ch in range(n_chunks):
                    b0 = ch * CHUNK
                    nb = min(CHUNK, nblk - b0)  # blocks in this chunk
                    n_ti = nb * TPB             # xi tiles in chunk
                    n_tl = nb * TL
                    n_th = nb * TH

                    # ---- per-chunk loads -------------------------------
                    ixi_t = idx_p.tile([128, CHUNK * TPB * 8], i16, tag="ixi")
                    ixlo_t = idx_p.tile([128, CHUNK * TL * 8], i16, tag="ixlo")
                    ixhi_t = idx_p.tile([128, CHUNK * TH * 8], i16, tag="ixhi")
                    ea_t = ea_p.tile([101, CHUNK * TPB * 128], bf, tag="ea")
                    c0 = b0 * TPB * 8
                    nc.sync.dma_start(ixi_t[:, :n_ti * 8],
                                      ixi_d[:, c0:c0 + n_ti * 8])
                    nc.sync.dma_start(ixlo_t[:, :n_tl * 8],
                                      ixlo_d[:, b0 * TL * 8: b0 * TL * 8 + n_tl * 8])
                    nc.sync.dma_start(ixhi_t[:, :n_th * 8],
                                      ixhi_d[:, b0 * TH * 8: b0 * TH * 8 + n_th * 8])
                    nc.sync.dma_start(ea_t[:, :n_ti * 128],
                                      eaT_d[:, b0 * TPB * 128: (b0 * TPB + n_ti) * 128])

                    # ---- gathers (SBUF-source, transposed, bf16) -------
                    xi_g = gxi_p.tile([128, 1, CHUNK * TPB * 128], bf, tag="xi")
                    lo_g = glo_p.tile([128, 1, CHUNK * TL * 128], bf, tag="lo")
                    hi_g = ghi_p.tile([128, 1, CHUNK * TH * 128], bf, tag="hi")
                    loc_view = loc_flat[:, b0 * 128:(b0 + nb) * 128]
                    ablate = _os.environ.get("ABLATE", "")
                    if "noxi" in ablate or "nogather" in ablate:
                        nc.vector.memset(xi_g[:], 0.25)
                    else:
                        nc.gpsimd.dma_gather(
                            xi_g[:, :, :n_ti * 128], loc_view, ixi_t[:, :n_ti * 8],
                            n_ti * 128, n_ti * 128, 128,
                            transpose=True, sbuf_tokens_per_rank=128,
                            sbuf_free_dim_per_rank=256, single_packet=False)
                    if "noxj" in ablate or "nogather" in ablate:
                        nc.vector.memset(lo_g[:], 0.25)
                        nc.vector.memset(hi_g[:], 0.25)
                    else:
                        nc.gpsimd.dma_gather(
                            lo_g[:, :, :n_tl * 128], tab_lo_view, ixlo_t[:, :n_tl * 8],
                            n_tl * 128, n_tl * 128, 128,
                            transpose=True, sbuf_tokens_per_rank=128,
                            sbuf_free_dim_per_rank=256, single_packet=False)
                        nc.gpsimd.dma_gather(
                            hi_g[:, :, :n_th * 128], tab_hi_view, ixhi_t[:, :n_th * 8],
                            n_th * 128, n_th * 128, 128,
                            transpose=True, sbuf_tokens_per_rank=128,
                            sbuf_free_dim_per_rank=256, single_packet=False)

                    # ---- per-block compute -----------------------------
                    for bi in range(nb):
                        blk = b0 + bi
                        agg = agg_p.tile([128, 128], f32, tag="agg")
                        for t in range(TPB):
                            is_lo = t < TL
                            xi_sl = xi_g[:, 0, (bi * TPB + t) * 128:
                                         (bi * TPB + t + 1) * 128]
                            if is_lo:
                                xj_sl = lo_g[:, 0, (bi * TL + t) * 128:
                                             (bi * TL + t + 1) * 128]
                            else:
                                th = t - TL
                                xj_sl = hi_g[:, 0, (bi * TH + th) * 128:
                                             (bi * TH + th + 1) * 128]
                            ea_sl = ea_t[:, (bi * TPB + t) * 128:
                                         (bi * TPB + t + 1) * 128]

                            zc = zc_p.tile([128, 256], f32, tag="zc")
                            nc.tensor.matmul(zc[:], xi_sl, wxi_s[:, layer, :],
                                             start=True, stop=False)
                            nc.tensor.matmul(zc[:], xj_sl, wxj_s[:, layer, :],
                                             start=False, stop=False)
                            nc.tensor.matmul(zc[:], ea_sl, wea_s[:, layer, :],
                                             start=False, stop=True)

                            sel = small_p.tile([128, 128], bf, tag="sel")
                            nc.vector.tensor_scalar(
                                out=sel[:], in0=iota_s[:],
                                scalar1=dst_s[:, blk * TPB + t: blk * TPB + t + 1],
                                scalar2=None, op0=mybir.AluOpType.is_equal)

                            # zc holds [-z1 | z2] (z1-half weights sign-flipped
                            # on host).  msg = softplus(z2) * sigmoid(z1)
                            #          = ln(1+e^{z2}) / (1 + e^{-z1})
                            ez = small_p.tile([128, 256], f32, tag="ez")
                            nc.scalar.activation(ez[:], zc[:],
                                                 mybir.ActivationFunctionType.Exp)
                            sp = small_p.tile([128, 128], bf, tag="sp")
                            nc.scalar.activation(sp[:], ez[:, 128:256],
                                                 mybir.ActivationFunctionType.Ln,
                                                 bias=ones_s[:])
                            u1 = small_p.tile([128, 128], f32, tag="u1")
                            nc.vector.tensor_scalar(
                                out=u1[:], in0=ez[:, 0:128], scalar1=1.0,
                                scalar2=None, op0=mybir.AluOpType.add)
                            rcp = small_p.tile([128, 128], f32, tag="rcp")
                            nc.vector.reciprocal(rcp[:], u1[:])
                            msg = small_p.tile([128, 128], bf, tag="msg")
                            nc.vector.tensor_mul(msg[:], sp[:], rcp[:])

                            nc.tensor.matmul(agg[:], sel[:], msg[:],
                                             start=(t == 0), stop=(t == TPB - 1))

                        # ---- block epilogue: LN + residual + softplus --
                        xold = xio_p.tile([128, 128], f32, tag="xold")
                        nc.sync.dma_start(
                            xold[:], xold_src[blk * 128:(blk + 1) * 128, :])

                        st = stats_p.tile([128, 6], f32, tag="bn")
                        nc.vector.bn_stats(out=st[:], in_=agg[:])
                        mv = stats_p.tile([128, 2], f32, tag="mv")
                        nc.vector.bn_aggr(out=mv[:], in_=st[:])
                        # rstd = exp(-0.5 * ln(var + eps))
                        lnv = stats_p.tile([128, 1], f32, tag="lnv")
                        nc.scalar.activation(lnv[:], mv[:, 1:2],
                                             mybir.ActivationFunctionType.Ln,
                                             bias=eps_s[:])
                        rstd = stats_p.tile([128, 1], f32, tag="rstd")
                        nc.scalar.activation(rstd[:], lnv[:],
                                             mybir.ActivationFunctionType.Exp,
                                             scale=-0.5)

                        xn = xio_p.tile([128, 128], f32, tag="xn")
                        nc.vector.tensor_scalar(
                            out=xn[:], in0=agg[:], scalar1=mv[:, 0:1],
                            scalar2=rstd[:], op0=mybir.AluOpType.subtract,
                            op1=mybir.AluOpType.mult)
                        nc.vector.tensor_mul(xn[:], xn[:], g_s[:, layer, :])
                        nc.vector.tensor_add(xn[:], xn[:], b_s[:, layer, :])
                        nc.vector.tensor_add(xn[:], xn[:], xold[:])

                        # softplus(xn) = ln(1 + e^{xn})
                        exn = xio_p.tile([128, 128], f32, tag="exn")
                        nc.scalar.activation(exn[:], xn[:],
                                             mybir.ActivationFunctionType.Exp)
                        xnew = xio_p.tile([128, 128], f32, tag="xnew")
                        nc.scalar.activation(xnew[:], exn[:],
                                             mybir.ActivationFunctionType.Ln,
                                             bias=ones_s[:])
                        # bf16 copy into the local gather table
                        nc.scalar.activation(loc_s[:, blk, :], xnew[:],
                                             mybir.ActivationFunctionType.Copy)
                        nc.sync.dma_start(
                            xnew_dst[blk * 128:(blk + 1) * 128, :], xnew[:])

                # ---- exchange (layers 0,1): slice -> AllGather -> table
                if layer < N_CONV - 1 and exchange != "none":
                    nc.sync.dma_start(
                        xout[layer].rearrange("(r p) c -> p r c", p=128),
                        loc_s[:])
                    nc.gpsimd.collective_compute(
                        "AllGather", mybir.AluOpType.bypass,
                        replica_groups=rg,
                        ins=[xout[layer][:]], outs=[xall[layer][:]])
                    nc.sync.dma_start(
                        tab_s[:].rearrange("p (r c) -> p r c", c=128),
                        xall[layer].rearrange("(r p) c -> p r c", p=128))

    nc.compile()
    return nc


# --------------------------------------------------------------------------
# Host preprocessing
# --------------------------------------------------------------------------

def _softplus(x):
    return np.log1p(np.exp(-np.abs(x))) + np.maximum(x, 0.0)


def preprocess(z, R, edge_index, embedding, emb_w, emb_b, conv_w, conv_b,
               ln_g, ln_b, n_nodes=N_NODES, n_cores=N_CORES, nblk=NBLK,
               lo_slots=LO_SLOTS, edge_d=EDGE_D):
    blocks = n_cores * nblk
    slots = blocks * 128
    core_slots = nblk * 128
    lo_blocks = lo_slots // 128
    n_edges = edge_index.shape[1]
    src = np.asarray(edge_index[0], np.int64)
    dst = np.asarray(edge_index[1], np.int64)

    # x0 on host
    EW = (np.asarray(embedding, np.float32) @ np.asarray(emb_w, np.float32)
          + np.asarray(emb_b, np.float32))
    x0 = EW[np.asarray(z, np.int64)]  # [N, 128] f32

    # edge smearing on host
    Rf = np.asarray(R, np.float32)
    d = np.linalg.norm(Rf[src] - Rf[dst], axis=-1)
    offs = np.linspace(0.0, CUTOFF, edge_d, dtype=np.float32)
    coeff = -0.5 / (offs[1] - offs[0]) ** 2
    ea = np.exp(coeff * (d[:, None] - offs[None, :]) ** 2)  # [E, 100] f32

    # node permutation: balance per-block in-degrees; L = orig nodes < lo_slots
    islo_e = src < lo_slots
    a = np.bincount(dst[islo_e], minlength=n_nodes)
    b = np.bincount(dst[~islo_e], minlength=n_nodes)
    w = a + b
    # L-nodes -> slots [0, lo_slots); rest -> [lo_slots, slots)
    ordL = np.argsort(-w[:lo_slots], kind="stable")
    ordH = np.argsort(-w[lo_slots:], kind="stable") + lo_slots
    perm = np.full(n_nodes, -1, np.int64)
    perm[ordL] = _snake_slots(ordL.size, lo_blocks)
    perm[ordH] = _snake_slots(ordH.size, blocks - lo_blocks) + lo_slots
    assert perm.min() >= 0

    es, ed = perm[src], perm[dst]
    blk = ed // 128

    lo_cnt = np.bincount(blk[islo_e], minlength=blocks)
    hi_cnt = np.bincount(blk[~islo_e], minlength=blocks)
    TL = int(-(-lo_cnt.max() // 128))
    TH = int(-(-hi_cnt.max() // 128))
    TPB = TL + TH
    S = nblk * TPB * 128

    # edge slot assignment: within block, lows first then highs
    key = blk * 2 + (~islo_e).astype(np.int64)
    eorder = np.argsort(key, kind="stable")
    ks = key[eorder]
    # position within each (block, half) run
    runstart = np.r_[0, np.flatnonzero(np.diff(ks)) + 1]
    runid = np.zeros(n_edges, np.int64)
    runid[runstart[1:]] = 1
    runid = np.cumsum(runid)
    pos = np.arange(n_edges) - runstart[runid]
    eb = ks // 2
    ehalf = ks % 2
    base = eb * TPB * 128 + ehalf * (TL * 128)
    eslot_g = base + pos                       # global edge slot (per full graph)
    # per-core arrays
    core_of = eb // nblk
    eslot = eslot_g - core_of * (nblk * TPB * 128)

    ixi = np.zeros((n_cores, S), np.int16)
    ixlo = np.zeros((n_cores, nblk * TL * 128), np.int16)
    ixhi = np.zeros((n_cores, nblk * TH * 128), np.int16)
    dstv = np.full((n_cores, nblk * TPB, 128), -1.0, np.float32)
    eaT = np.zeros((n_cores, 101, S), BF16)

    e_src = es[eorder]
    e_dst = ed[eorder]
    e_lo = ehalf == 0
    ea_o = ea[eorder]

    for c in range(n_cores):
        m = core_of == c
        sl = eslot[m]
        # xi: dst local to the chunk's 2-block view
        dloc = (e_dst[m] - c * core_slots) % (CHUNK * 128)
        ixi[c][sl] = dloc.astype(np.int16)
        # xj
        mlo = m & e_lo
        mhi = m & ~e_lo
        slo_ = eslot[mlo]
        # map edge slot -> position in the lo stream
        bb = slo_ // (TPB * 128)
        off = slo_ - bb * (TPB * 128)
        ixlo[c][bb * TL * 128 + off] = e_src[mlo].astype(np.int16)
        shi_ = eslot[mhi]
        bb = shi_ // (TPB * 128)
        off = shi_ - bb * (TPB * 128) - TL * 128
        ixhi[c][bb * TH * 128 + off] = (e_src[mhi] - lo_slots).astype(np.int16)
        # dst one-hot value and edge features
        dstv[c].reshape(-1)[sl] = (e_dst[m] % 128).astype(np.float32)
        eaT[c][:edge_d, sl] = ea_o[m].T.astype(BF16)
        eaT[c][100, sl] = np.float32(1.0).astype(BF16)

    def wrap16(arr):
        # [S] int16 -> [128, S/16], idx i at (i%16, i//16), tiled to 128 parts
        t = arr.reshape(-1, 16).T
        return np.tile(t, (8, 1)).copy()

    # initial tables
    x0s = np.zeros((slots, 128), np.float32)
    inv = np.full(slots, -1, np.int64)
    inv[perm] = np.arange(n_nodes)
    valid = inv >= 0
    x0s[valid] = x0[inv[valid]]
    x0b = x0s.astype(BF16)
    tab0 = np.ascontiguousarray(
        x0b.reshape(blocks, 128, 128).transpose(1, 0, 2).reshape(128, blocks * 128))

    # weights; z1-half output columns sign-flipped so the device computes
    # [-z1 | z2] and can use exp/ln-only activations
    cw = np.asarray(conv_w, np.float32).copy()
    cb = np.asarray(conv_b, np.float32).copy()
    cw[:, :, :128] *= -1.0
    cb[:, :128] *= -1.0
    wxi = np.ascontiguousarray(cw[:, :128, :].transpose(1, 0, 2)).astype(BF16)
    wxj = np.ascontiguousarray(cw[:, 128:256, :].transpose(1, 0, 2)).astype(BF16)
    wea = np.concatenate([cw[:, 256:, :], cb[:, None, :]], axis=1)
    wea = np.ascontiguousarray(wea.transpose(1, 0, 2)).astype(BF16)
    lg = np.tile(np.asarray(ln_g, np.float32)[None, :, :], (128, 1, 1))
    lb = np.tile(np.asarray(ln_b, np.float32)[None, :, :], (128, 1, 1))
    iota = np.tile(np.arange(128, dtype=np.float32)[None, :], (128, 1))

    in_maps = []
    for c in range(n_cores):
        sl0 = c * core_slots
        loc0 = np.ascontiguousarray(
            x0b[sl0:sl0 + core_slots].reshape(nblk, 128, 128)
            .transpose(1, 0, 2).reshape(128, nblk * 128))
        in_maps.append({
            "tab0": tab0,
            "loc0": loc0,
            "xloc0": np.ascontiguousarray(x0s[sl0:sl0 + core_slots]),
            "eaT": np.ascontiguousarray(eaT[c]),
            "ixi": wrap16(ixi[c]),
            "ixlo": wrap16(ixlo[c]),
            "ixhi": wrap16(ixhi[c]),
            "dstv": np.ascontiguousarray(dstv[c].transpose(1, 0)),
            "iota": iota,
            "wxi": wxi, "wxj": wxj, "wea": wea,
            "lng": lg, "lnb": lb,
        })
    return in_maps, perm, TL, TH


def _snake_slots(n, n_bins):
    """Slot offsets (bin*128 + round) for n items dealt snake-wise, in the
    order of the sorted item list."""
    idx = np.arange(n)
    r = idx // n_bins
    k = idx % n_bins
    bins = np.where(r % 2 == 0, k, n_bins - 1 - k)
    return bins * 128 + r


# --------------------------------------------------------------------------
# kernel entry
# --------------------------------------------------------------------------

def kernel(z, R, edge_index, batch, embedding, emb_w, emb_b, conv_w, conv_b,
           ln_g, ln_b, cfc_w, cfc_b, fc_w, fc_b, out_w, out_b):
    in_maps, perm, TL, TH = preprocess(
        z, R, edge_index, embedding, emb_w, emb_b, conv_w, conv_b, ln_g, ln_b)

    nc = build_nc(TL, TH)
    res = run_bass_kernel_spmd(nc, in_maps, core_ids=list(range(N_CORES)))
    global LAST_RESULTS, LAST_RERUN_S
    LAST_RESULTS = res
    if _os.environ.get("KERNEL_RERUN"):
        import time as _time
        t0 = _time.time()
        run_bass_kernel_spmd(nc, in_maps, core_ids=list(range(N_CORES)))
        LAST_RERUN_S = _time.time() - t0

    xs = np.concatenate([res.results[c]["xfin"] for c in range(N_CORES)], axis=0)
    x = xs[perm]  # [N_NODES, 128] f32, original node order

    batch = np.asarray(batch, np.int64)
    sums = np.zeros((N_GRAPHS, NODE_D), np.float32)
    np.add.at(sums, batch, x)
    cnts = np.bincount(batch, minlength=N_GRAPHS).astype(np.float32)
    mol = sums / np.maximum(cnts, 1.0)[:, None]

    h = _softplus(mol @ np.asarray(cfc_w, np.float32) + np.asarray(cfc_b, np.float32))
    for l in range(np.asarray(fc_w).shape[0]):
        h = _softplus(h @ np.asarray(fc_w[l], np.float32)
                      + np.asarray(fc_b[l], np.float32))
    out = h @ np.asarray(out_w, np.float32) + np.asarray(out_b, np.float32)
    return out.astype(np.float32)
